# revision 3
# baseline (speedup 1.0000x reference)
"""
Trainium2 Bass kernel for nn_BMM_S8T_S8N_S8T:
  y[b,m,n] = sat_i8(round(alpha * sum_k a[b,m,k] * b[b,n,k]))
with a,b int8 [128, 1024, 128], alpha scalar.

Strategy (8 NeuronCores, batch-parallel, 16 batches/core):
 - Host: pre-transpose a -> [BPC, K, M], b -> [BPC, K, N] so SBUF tiles land
   directly in [contraction-partition, free] layout. No on-chip transposes.
 - Input DMA on SWDGE (gpsimd) casts int8 -> bf16 in the DMA datapath; the
   engines never touch input conversion. bf16 holds int8 exactly; products
   (<= 2^14) and fp32 accumulations (|acc| <= 2^21) are bit-exact.
 - Matmuls: per batch, 8 stationary A-tiles [128k, 128m] x moving B [128k, 512n]
   pairs into [128, 2048] fp32 PSUM tiles (4 banks, double-buffered).
 - Epilogue: one op per PSUM tile: int8 out = rne_sat(alpha*acc), alternating
   ACT (activation Copy w/ scale) and DVE (tensor_scalar mult) in a 5:4
   pattern that balances the 1.2 GHz vs 0.96 GHz engines. This drain is the
   critical path (~64 us); everything else hides under it.
 - Stores: one 1 MiB DMA per batch, alternating the two HWDGE rings.
"""

import sys

sys.path.insert(0, "/opt/trn_rl_repo")

import numpy as np

N_CORES = 8
B, M, N, K = 128, 1024, 1024, 128
BPC = B // N_CORES  # batches per core
MT = M // 128
HALF = BPC // 2

_cache = {}


def _build(alpha: float):
    import concourse.bacc as bacc
    import concourse.tile as tile
    import concourse.mybir as mybir

    nc = bacc.Bacc("TRN2", target_bir_lowering=False, debug=False)

    a_t = nc.dram_tensor("a_t", [BPC, K, M], mybir.dt.int8, kind="ExternalInput")
    b_t = nc.dram_tensor("b_t", [BPC, K, N], mybir.dt.int8, kind="ExternalInput")
    y = nc.dram_tensor("y", [BPC, M, N], mybir.dt.int8, kind="ExternalOutput")

    bf16 = mybir.dt.bfloat16
    f32 = mybir.dt.float32
    i8 = mybir.dt.int8

    a_v = a_t.rearrange("b k m -> k b m")  # [128, BPC, 1024]
    b_v = b_t.rearrange("b k n -> k b n")

    with tile.TileContext(nc) as tc:
        with (
            tc.tile_pool(name="inp", bufs=1) as ipool,
            tc.tile_pool(name="outp", bufs=3) as opool,
            tc.tile_pool(name="ps", bufs=2, space="PSUM") as pspool,
        ):
            # input tiles: all 16 batches resident as bf16 (64 KB/partition)
            ab = {}
            for h in range(2):
                ta = ipool.tile([128, HALF, M], bf16, tag=f"a{h}")
                tb = ipool.tile([128, HALF, N], bf16, tag=f"b{h}")
                ab["a", h] = ta
                ab["b", h] = tb

            def load_half(h):
                lo, hi = h * HALF, (h + 1) * HALF
                nc.gpsimd.dma_start(out=ab["a", h][:], in_=a_v[:, lo:hi, :])
                nc.gpsimd.dma_start(out=ab["b", h][:], in_=b_v[:, lo:hi, :])

            load_half(0)
            load_half(1)

            di = 0  # drain-op index for ACT/DVE balancing
            for bi in range(BPC):
                h, r = divmod(bi, HALF)
                at = ab["a", h][:, r, :]  # [128, 1024] k x m
                bt = ab["b", h][:, r, :]  # [128, 1024] k x n
                y_sb = opool.tile([128, MT, N], i8, tag="y")

                for q in range(4):  # psum-tile quarter: m-tiles (2q, 2q+1)
                    ps = pspool.tile([128, 2, 2, 512], f32, tag="ps")
                    for t in range(2):
                        mt = 2 * q + t
                        for nh in range(2):
                            nc.tensor.matmul(
                                ps[:, t, nh, :],
                                at[:, mt * 128 : (mt + 1) * 128],
                                bt[:, nh * 512 : (nh + 1) * 512],
                                start=True,
                                stop=True,
                            )
                    dst = y_sb[:, 2 * q : 2 * q + 2, :].rearrange("p t n -> p (t n)")
                    src = ps.rearrange("p t nh n -> p (t nh n)")
                    if di % 9 % 2 == 0:  # 5 of 9 -> ACT, 4 of 9 -> DVE
                        nc.scalar.activation(
                            out=dst,
                            in_=src,
                            func=mybir.ActivationFunctionType.Copy,
                            scale=float(alpha),
                        )
                    else:
                        nc.vector.tensor_scalar(
                            out=dst,
                            in0=src,
                            scalar1=float(alpha),
                            scalar2=None,
                            op0=mybir.AluOpType.mult,
                        )
                    di += 1

                store_eng = nc.sync if bi % 2 == 0 else nc.scalar
                store_eng.dma_start(
                    out=y[bi].rearrange("(t p) n -> p t n", p=128), in_=y_sb[:]
                )

    nc.compile()
    return nc


def _get_nc(alpha: float):
    key = float(alpha)
    if key not in _cache:
        _cache[key] = _build(key)
    return _cache[key]


def _shard_inputs(a, b):
    # host-side pre-transpose to [B, K, M] / [B, K, N]
    a_t = np.ascontiguousarray(a.transpose(0, 2, 1))
    b_t = np.ascontiguousarray(b.transpose(0, 2, 1))
    return [
        {
            "a_t": a_t[c * BPC : (c + 1) * BPC],
            "b_t": b_t[c * BPC : (c + 1) * BPC],
        }
        for c in range(N_CORES)
    ]


def kernel(a, b, alpha):
    from concourse.bass_utils import run_bass_kernel_spmd

    a = np.asarray(a)
    b = np.asarray(b)
    assert a.shape == (B, M, K) and a.dtype == np.int8
    assert b.shape == (B, N, K) and b.dtype == np.int8

    nc = _get_nc(float(alpha))
    in_maps = _shard_inputs(a, b)
    res = run_bass_kernel_spmd(nc, in_maps, list(range(N_CORES)))
    out = np.concatenate([r["y"] for r in res.results], axis=0)
    return out.astype(np.int8)


# revision 5
# speedup vs baseline: 1.3245x; 1.3245x over previous
"""
Trainium2 Bass kernel for nn_BMM_S8T_S8N_S8T:
  y[b,m,n] = sat_i8(round(alpha * sum_k a[b,m,k] * b[b,n,k]))
with a,b int8 [128, 1024, 128], alpha scalar.

Strategy (8 NeuronCores, batch-parallel, 16 batches/core):
 - Host: pre-transpose a -> [BPC, K, M], b -> [BPC, K, N] so SBUF tiles land
   directly in [contraction-partition, free] layout. No on-chip transposes.
 - Input DMA on SWDGE (gpsimd) casts int8 -> bf16 in the DMA datapath; the
   engines never touch input conversion. bf16 holds int8 exactly; products
   (<= 2^14) and fp32 accumulations (|acc| <= 2^21) are bit-exact.
 - Matmuls: per batch, 8 stationary A-tiles [128k, 128m] x moving B [128k, 512n]
   pairs into [128, 2048] fp32 PSUM tiles (4 banks, double-buffered).
 - Epilogue: one op per PSUM tile: int8 out = rne_sat(alpha*acc), alternating
   ACT (activation Copy w/ scale) and DVE (tensor_scalar mult) in a 5:4
   pattern that balances the 1.2 GHz vs 0.96 GHz engines. This drain is the
   critical path (~64 us); everything else hides under it.
 - Stores: one 1 MiB DMA per batch, alternating the two HWDGE rings.
"""

import sys

sys.path.insert(0, "/opt/trn_rl_repo")

import numpy as np

N_CORES = 8
B, M, N, K = 128, 1024, 1024, 128
BPC = B // N_CORES  # batches per core
MT = M // 128
HALF = BPC // 2

_cache = {}


def _build(alpha: float):
    import concourse.bacc as bacc
    import concourse.tile as tile
    import concourse.mybir as mybir

    nc = bacc.Bacc("TRN2", target_bir_lowering=False, debug=False)

    a_t = nc.dram_tensor("a_t", [BPC, K, M], mybir.dt.int8, kind="ExternalInput")
    b_t = nc.dram_tensor("b_t", [BPC, K, N], mybir.dt.int8, kind="ExternalInput")
    y = nc.dram_tensor("y", [BPC, M, N], mybir.dt.int8, kind="ExternalOutput")

    bf16 = mybir.dt.bfloat16
    f32 = mybir.dt.float32
    i8 = mybir.dt.int8

    a_v = a_t.rearrange("b k m -> k b m")  # [128, BPC, 1024]
    b_v = b_t.rearrange("b k n -> k b n")

    with tile.TileContext(nc) as tc:
        with (
            tc.tile_pool(name="inp", bufs=1) as ipool,
            tc.tile_pool(name="outp", bufs=3) as opool,
            tc.tile_pool(name="ps", bufs=4, space="PSUM") as pspool,
        ):
            # input tiles: all 16 batches resident as bf16 (64 KB/partition)
            ab = {}
            for h in range(2):
                ta = ipool.tile([128, HALF, M], bf16, tag=f"a{h}")
                tb = ipool.tile([128, HALF, N], bf16, tag=f"b{h}")
                ab["a", h] = ta
                ab["b", h] = tb

            # chunked cast loads: 2+2+4+4+4 batches per tensor so batch 0's
            # inputs land quickly and compute starts early
            chunks = [(0, 2), (2, 4), (4, 8), (8, 12), (12, 16)]
            for lo, hi in chunks:
                h = 0 if lo < HALF else 1
                s, e = lo - h * HALF, hi - h * HALF
                nc.gpsimd.dma_start(out=ab["a", h][:, s:e, :], in_=a_v[:, lo:hi, :])
                nc.gpsimd.dma_start(out=ab["b", h][:, s:e, :], in_=b_v[:, lo:hi, :])

            di = 0  # drain-op index for ACT/DVE balancing
            for bi in range(BPC):
                h, r = divmod(bi, HALF)
                at = ab["a", h][:, r, :]  # [128, 1024] k x m
                bt = ab["b", h][:, r, :]  # [128, 1024] k x n
                y_sb = opool.tile([128, MT, N], i8, tag="y")

                for mt in range(MT):  # one 2-bank psum tile per m-tile
                    ps = pspool.tile([128, 2, 512], f32, tag="ps")
                    for nh in range(2):
                        nc.tensor.matmul(
                            ps[:, nh, :],
                            at[:, mt * 128 : (mt + 1) * 128],
                            bt[:, nh * 512 : (nh + 1) * 512],
                            start=True,
                            stop=True,
                        )
                    dst = y_sb[:, mt, :]
                    src = ps.rearrange("p a b -> p (a b)")
                    # strict A/D interleave over an odd cycle: 9 ACT, 8 DVE
                    # per 17 tiles (balances 1.2 GHz ACT vs 0.96 GHz DVE)
                    if di % 17 % 2 == 0:
                        nc.scalar.activation(
                            out=dst,
                            in_=src,
                            func=mybir.ActivationFunctionType.Copy,
                            scale=float(alpha),
                        )
                    else:
                        nc.vector.tensor_scalar(
                            out=dst,
                            in0=src,
                            scalar1=float(alpha),
                            scalar2=None,
                            op0=mybir.AluOpType.mult,
                        )
                    di += 1

                store_eng = nc.sync if bi % 2 == 0 else nc.scalar
                store_eng.dma_start(
                    out=y[bi].rearrange("(t p) n -> p t n", p=128), in_=y_sb[:]
                )

    nc.compile()
    return nc


def _get_nc(alpha: float):
    key = float(alpha)
    if key not in _cache:
        _cache[key] = _build(key)
    return _cache[key]


def _shard_inputs(a, b):
    # host-side pre-transpose to [B, K, M] / [B, K, N]
    a_t = np.ascontiguousarray(a.transpose(0, 2, 1))
    b_t = np.ascontiguousarray(b.transpose(0, 2, 1))
    return [
        {
            "a_t": a_t[c * BPC : (c + 1) * BPC],
            "b_t": b_t[c * BPC : (c + 1) * BPC],
        }
        for c in range(N_CORES)
    ]


def kernel(a, b, alpha):
    from concourse.bass_utils import run_bass_kernel_spmd

    a = np.asarray(a)
    b = np.asarray(b)
    assert a.shape == (B, M, K) and a.dtype == np.int8
    assert b.shape == (B, N, K) and b.dtype == np.int8

    nc = _get_nc(float(alpha))
    in_maps = _shard_inputs(a, b)
    res = run_bass_kernel_spmd(nc, in_maps, list(range(N_CORES)))
    out = np.concatenate([r["y"] for r in res.results], axis=0)
    return out.astype(np.int8)


# revision 8
# speedup vs baseline: 1.4227x; 1.0742x over previous
"""
Trainium2 Bass kernel for nn_BMM_S8T_S8N_S8T:
  y[b,m,n] = sat_i8(round(alpha * sum_k a[b,m,k] * b[b,n,k]))
with a,b int8 [128, 1024, 128], alpha scalar.

Strategy (8 NeuronCores, batch-parallel, 16 batches/core):
 - Host: pre-transpose a -> [BPC, K, M], b -> [BPC, K, N] so SBUF tiles land
   directly in [contraction-partition, free] layout. No on-chip transposes.
 - Input DMA on SWDGE (gpsimd) casts int8 -> bf16 in the DMA datapath; the
   engines never touch input conversion. bf16 holds int8 exactly; products
   (<= 2^14) and fp32 accumulations (|acc| <= 2^21) are bit-exact.
 - Matmuls: per batch, 8 stationary A-tiles [128k, 128m] x moving B [128k, 512n]
   pairs into [128, 2048] fp32 PSUM tiles (4 banks, double-buffered).
 - Epilogue: one op per PSUM tile: int8 out = rne_sat(alpha*acc), alternating
   ACT (activation Copy w/ scale) and DVE (tensor_scalar mult) in a 5:4
   pattern that balances the 1.2 GHz vs 0.96 GHz engines. This drain is the
   critical path (~64 us); everything else hides under it.
 - Stores: one 1 MiB DMA per batch, alternating the two HWDGE rings.
"""

import sys

sys.path.insert(0, "/opt/trn_rl_repo")

import numpy as np

N_CORES = 8
B, M, N, K = 128, 1024, 1024, 128
BPC = B // N_CORES  # batches per core
MT = M // 128
HALF = BPC // 2

_cache = {}


def _build(alpha: float):
    import concourse.bacc as bacc
    import concourse.tile as tile
    import concourse.mybir as mybir

    nc = bacc.Bacc("TRN2", target_bir_lowering=False, debug=False)

    a_t = nc.dram_tensor("a_t", [BPC, K, M], mybir.dt.int8, kind="ExternalInput")
    b_t = nc.dram_tensor("b_t", [BPC, K, N], mybir.dt.int8, kind="ExternalInput")
    y = nc.dram_tensor("y", [BPC, M, N], mybir.dt.int8, kind="ExternalOutput")

    bf16 = mybir.dt.bfloat16
    f32 = mybir.dt.float32
    i8 = mybir.dt.int8

    a_v = a_t.rearrange("b k m -> k b m")  # [128, BPC, 1024]
    b_v = b_t.rearrange("b k n -> k b n")

    with tile.TileContext(nc) as tc:
        with (
            tc.tile_pool(name="inp", bufs=1) as ipool,
            tc.tile_pool(name="outp", bufs=3) as opool,
            tc.tile_pool(name="ps", bufs=4, space="PSUM") as pspool,
        ):
            # input tiles: all 16 batches resident as bf16 (64 KB/partition)
            ab = {}
            for h in range(2):
                ta = ipool.tile([128, HALF, M], bf16, tag=f"a{h}")
                tb = ipool.tile([128, HALF, N], bf16, tag=f"b{h}")
                ab["a", h] = ta
                ab["b", h] = tb

            # Batches 0-3: raw int8 via the fast HWDGE queues + conversion on
            # ACT/DVE while they are otherwise idle (prologue). Batches 4-15:
            # SWDGE cast-DMA (int8->bf16 in the DMA datapath) in 4 big chunks
            # to amortize the ~2.7us per-DMA Q7 issue cost.
            raws = []
            for pr in range(2):  # batch pairs (0,1) and (2,3)
                ra = ipool.tile([128, 2, M], i8, tag=f"ra{pr}")
                rb = ipool.tile([128, 2, N], i8, tag=f"rb{pr}")
                nc.sync.dma_start(out=ra[:], in_=a_v[:, 2 * pr : 2 * pr + 2, :])
                nc.scalar.dma_start(out=rb[:], in_=b_v[:, 2 * pr : 2 * pr + 2, :])
                raws.append((ra, rb))
            nc.gpsimd.dma_start(out=ab["a", 0][:, 4:8, :], in_=a_v[:, 4:8, :])
            nc.gpsimd.dma_start(out=ab["b", 0][:, 4:8, :], in_=b_v[:, 4:8, :])
            nc.gpsimd.dma_start(out=ab["a", 1][:], in_=a_v[:, 8:16, :])
            nc.gpsimd.dma_start(out=ab["b", 1][:], in_=b_v[:, 8:16, :])
            for pr in range(2):
                ra, rb = raws[pr]
                sl = slice(2 * pr, 2 * pr + 2)
                nc.vector.tensor_copy(out=ab["a", 0][:, sl, :], in_=ra[:])
                nc.scalar.activation(
                    out=ab["b", 0][:, sl, :],
                    in_=rb[:],
                    func=mybir.ActivationFunctionType.Copy,
                    scale=1.0,
                )

            di = 0  # drain-op index for ACT/DVE balancing
            for bi in range(BPC):
                h, r = divmod(bi, HALF)
                at = ab["a", h][:, r, :]  # [128, 1024] k x m
                bt = ab["b", h][:, r, :]  # [128, 1024] k x n
                y_sb = opool.tile([128, MT, N], i8, tag="y")

                for mt in range(MT):  # one 2-bank psum tile per m-tile
                    ps = pspool.tile([128, 2, 512], f32, tag="ps")
                    for nh in range(2):
                        nc.tensor.matmul(
                            ps[:, nh, :],
                            at[:, mt * 128 : (mt + 1) * 128],
                            bt[:, nh * 512 : (nh + 1) * 512],
                            start=True,
                            stop=True,
                        )
                    dst = y_sb[:, mt, :]
                    src = ps.rearrange("p a b -> p (a b)")
                    # strict A/D interleave over an odd cycle: 9 ACT, 8 DVE
                    # per 17 tiles (balances 1.2 GHz ACT vs 0.96 GHz DVE)
                    if di % 17 % 2 == 0:
                        nc.scalar.activation(
                            out=dst,
                            in_=src,
                            func=mybir.ActivationFunctionType.Copy,
                            scale=float(alpha),
                        )
                    else:
                        nc.vector.tensor_scalar(
                            out=dst,
                            in0=src,
                            scalar1=float(alpha),
                            scalar2=None,
                            op0=mybir.AluOpType.mult,
                        )
                    di += 1
                    # half-batch stores (512 KiB) as soon as each half is
                    # drained, alternating HWDGE rings: smooth store flow and
                    # a short end-of-kernel tail
                    if mt == 3:
                        nc.sync.dma_start(
                            out=y[bi].rearrange("(t p) n -> p t n", p=128)[:, 0:4, :],
                            in_=y_sb[:, 0:4, :],
                        )
                    elif mt == 7:
                        nc.scalar.dma_start(
                            out=y[bi].rearrange("(t p) n -> p t n", p=128)[:, 4:8, :],
                            in_=y_sb[:, 4:8, :],
                        )

    nc.compile()
    return nc


def _get_nc(alpha: float):
    key = float(alpha)
    if key not in _cache:
        _cache[key] = _build(key)
    return _cache[key]


def _shard_inputs(a, b):
    # host-side pre-transpose to [B, K, M] / [B, K, N]
    a_t = np.ascontiguousarray(a.transpose(0, 2, 1))
    b_t = np.ascontiguousarray(b.transpose(0, 2, 1))
    return [
        {
            "a_t": a_t[c * BPC : (c + 1) * BPC],
            "b_t": b_t[c * BPC : (c + 1) * BPC],
        }
        for c in range(N_CORES)
    ]


def kernel(a, b, alpha):
    from concourse.bass_utils import run_bass_kernel_spmd

    a = np.asarray(a)
    b = np.asarray(b)
    assert a.shape == (B, M, K) and a.dtype == np.int8
    assert b.shape == (B, N, K) and b.dtype == np.int8

    nc = _get_nc(float(alpha))
    in_maps = _shard_inputs(a, b)
    res = run_bass_kernel_spmd(nc, in_maps, list(range(N_CORES)))
    out = np.concatenate([r["y"] for r in res.results], axis=0)
    return out.astype(np.int8)


# revision 13
# speedup vs baseline: 1.6630x; 1.1689x over previous
"""
Trainium2 Bass kernel for nn_BMM_S8T_S8N_S8T:
  y[b,m,n] = sat_i8(round(alpha * sum_k a[b,m,k] * b[b,n,k]))
with a,b int8 [128, 1024, 128], alpha scalar.

Strategy (8 NeuronCores, batch-parallel, 16 batches/core):
 - Host: pre-transpose a -> [BPC, K, M], b -> [BPC, K, N] so SBUF tiles land
   directly in [contraction-partition, free] layout. No on-chip transposes.
 - Input DMA on SWDGE (gpsimd) casts int8 -> bf16 in the DMA datapath; the
   engines never touch input conversion. bf16 holds int8 exactly; products
   (<= 2^14) and fp32 accumulations (|acc| <= 2^21) are bit-exact.
 - Matmuls: per batch, 8 stationary A-tiles [128k, 128m] x moving B [128k, 512n]
   pairs into [128, 2048] fp32 PSUM tiles (4 banks, double-buffered).
 - Epilogue: one op per PSUM tile: int8 out = rne_sat(alpha*acc), alternating
   ACT (activation Copy w/ scale) and DVE (tensor_scalar mult) in a 5:4
   pattern that balances the 1.2 GHz vs 0.96 GHz engines. This drain is the
   critical path (~64 us); everything else hides under it.
 - Stores: one 1 MiB DMA per batch, alternating the two HWDGE rings.
"""

import sys

sys.path.insert(0, "/opt/trn_rl_repo")

import numpy as np

N_CORES = 8
B, M, N, K = 128, 1024, 1024, 128
BPC = B // N_CORES  # batches per core
MT = M // 128
HALF = BPC // 2
NBF = 3  # leading batches shipped as host-prepared bf16 (prologue fast path)

_cache = {}


def _build(alpha: float):
    import concourse.bacc as bacc
    import concourse.tile as tile
    import concourse.mybir as mybir

    nc = bacc.Bacc("TRN2", target_bir_lowering=False, debug=False)

    a_t = nc.dram_tensor("a_t", [BPC, K, M], mybir.dt.int8, kind="ExternalInput")
    b_t = nc.dram_tensor("b_t", [BPC, K, N], mybir.dt.int8, kind="ExternalInput")
    # host-prepared bf16 copies of the first NBF batches (prologue fast path:
    # HWDGE loads them directly, engines never do input conversion)
    a_bf = nc.dram_tensor("a_bf", [NBF, K, M], mybir.dt.bfloat16, kind="ExternalInput")
    b_bf = nc.dram_tensor("b_bf", [NBF, K, N], mybir.dt.bfloat16, kind="ExternalInput")
    y = nc.dram_tensor("y", [BPC, M, N], mybir.dt.int8, kind="ExternalOutput")

    bf16 = mybir.dt.bfloat16
    f32 = mybir.dt.float32
    i8 = mybir.dt.int8

    a_v = a_t.rearrange("b k m -> k b m")  # [128, BPC, 1024]
    b_v = b_t.rearrange("b k n -> k b n")

    with tile.TileContext(nc) as tc:
        with (
            tc.tile_pool(name="inp", bufs=1) as ipool,
            tc.tile_pool(name="outp", bufs=3) as opool,
            tc.tile_pool(name="ps", bufs=4, space="PSUM") as pspool,
        ):
            # input tiles: all 16 batches resident as bf16 (64 KB/partition)
            ab = {}
            for h in range(2):
                ta = ipool.tile([128, HALF, M], bf16, tag=f"a{h}")
                tb = ipool.tile([128, HALF, N], bf16, tag=f"b{h}")
                ab["a", h] = ta
                ab["b", h] = tb

            # Batches 0..NBF-1: host-prepared bf16 via the two HWDGE rings in
            # parallel (no cast, no engine work). Batches NBF-15: SWDGE
            # cast-DMA (int8->bf16 in the DMA datapath) in 4 big chunks to
            # amortize the ~2.7us per-DMA Q7 issue cost.
            nc.sync.dma_start(
                out=ab["a", 0][:, 0:NBF, :], in_=a_bf.rearrange("b k m -> k b m")
            )
            nc.scalar.dma_start(
                out=ab["b", 0][:, 0:NBF, :], in_=b_bf.rearrange("b k n -> k b n")
            )
            nc.gpsimd.dma_start(out=ab["a", 0][:, NBF:8, :], in_=a_v[:, NBF:8, :])
            nc.gpsimd.dma_start(out=ab["b", 0][:, NBF:8, :], in_=b_v[:, NBF:8, :])
            nc.gpsimd.dma_start(out=ab["a", 1][:], in_=a_v[:, 8:16, :])
            nc.gpsimd.dma_start(out=ab["b", 1][:], in_=b_v[:, 8:16, :])

            di = 0  # drain-op index for ACT/DVE balancing
            for bi in range(BPC):
                h, r = divmod(bi, HALF)
                at = ab["a", h][:, r, :]  # [128, 1024] k x m
                bt = ab["b", h][:, r, :]  # [128, 1024] k x n
                y_sb = opool.tile([128, MT, N], i8, tag="y")

                for mt in range(MT):  # one 2-bank psum tile per m-tile
                    ps = pspool.tile([128, 2, 512], f32, tag="ps")
                    for nh in range(2):
                        nc.tensor.matmul(
                            ps[:, nh, :],
                            at[:, mt * 128 : (mt + 1) * 128],
                            bt[:, nh * 512 : (nh + 1) * 512],
                            start=True,
                            stop=True,
                        )
                    dst = y_sb[:, mt, :]
                    src = ps.rearrange("p a b -> p (a b)")
                    # strict A/D interleave over an odd cycle: 9 ACT, 8 DVE
                    # per 17 tiles (balances 1.2 GHz ACT vs 0.96 GHz DVE)
                    if di % 17 % 2 == 0:
                        nc.scalar.activation(
                            out=dst,
                            in_=src,
                            func=mybir.ActivationFunctionType.Copy,
                            scale=float(alpha),
                        )
                    else:
                        nc.vector.tensor_scalar(
                            out=dst,
                            in0=src,
                            scalar1=float(alpha),
                            scalar2=None,
                            op0=mybir.AluOpType.mult,
                        )
                    di += 1
                    # half-batch stores (512 KiB) as soon as each half is
                    # drained, all on the otherwise-idle sync HWDGE ring (the
                    # scalar ring would steal ACT sequencer time). Last batch
                    # stores in quarters to shrink the end-of-kernel tail.
                    yv = y[bi].rearrange("(t p) n -> p t n", p=128)
                    last = bi == BPC - 1
                    if mt == 3 and not last:
                        nc.sync.dma_start(out=yv[:, 0:4, :], in_=y_sb[:, 0:4, :])
                    elif mt == 7 and not last:
                        nc.sync.dma_start(out=yv[:, 4:8, :], in_=y_sb[:, 4:8, :])
                    elif last and mt % 2 == 1:
                        sl = slice(mt - 1, mt + 1)
                        nc.sync.dma_start(out=yv[:, sl, :], in_=y_sb[:, sl, :])

    nc.compile()
    return nc


def _get_nc(alpha: float):
    key = float(alpha)
    if key not in _cache:
        _cache[key] = _build(key)
    return _cache[key]


def _shard_inputs(a, b):
    import ml_dtypes

    # host-side pre-transpose to [B, K, M] / [B, K, N]
    a_t = np.ascontiguousarray(a.transpose(0, 2, 1))
    b_t = np.ascontiguousarray(b.transpose(0, 2, 1))
    maps = []
    for c in range(N_CORES):
        at = a_t[c * BPC : (c + 1) * BPC]
        bt = b_t[c * BPC : (c + 1) * BPC]
        maps.append(
            {
                "a_t": at,
                "b_t": bt,
                "a_bf": at[:NBF].astype(ml_dtypes.bfloat16),
                "b_bf": bt[:NBF].astype(ml_dtypes.bfloat16),
            }
        )
    return maps


def kernel(a, b, alpha):
    from concourse.bass_utils import run_bass_kernel_spmd

    a = np.asarray(a)
    b = np.asarray(b)
    assert a.shape == (B, M, K) and a.dtype == np.int8
    assert b.shape == (B, N, K) and b.dtype == np.int8

    nc = _get_nc(float(alpha))
    in_maps = _shard_inputs(a, b)
    res = run_bass_kernel_spmd(nc, in_maps, list(range(N_CORES)))
    out = np.concatenate([r["y"] for r in res.results], axis=0)
    return out.astype(np.int8)
